# revision 22
# baseline (speedup 1.0000x reference)
"""Trainium2 Bass kernel for nn_ClockAwareGNN (segment_reduce).

Model (reference, fp32):
    gp   = segment_mean(x, batch) @ W_base + b_base            # [B, 1]
    h    = relu(clock @ W1 + b1) @ W2 + b2                     # [N, 16]
    cp   = segment_mean(h, batch)                              # [B, 16]
    out  = relu([gp | cp] @ W3 + b3) @ W4 + b4                 # [B, 1]

Everything after the segment reductions is affine in per-graph quantities, so
the heavy per-node work collapses to fused segment sums:
    Sx[g] = sum of x rows in graph g           (128 cols, fp8 payload)
    Sr[g] = sum of r rows in graph g           (R cols, fp8 hi + fp8 lo*512)
where r is either the raw clock (R=1; exact when b1 == 0 and clock >= 0 since
relu(c*W1) == c*relu(W1) elementwise for c >= 0) or the host-computed
relu(clock @ W1 + b1) (R=16 fallback). Graph node counts come from `batch` on
the host (they are index metadata), shipped as a per-graph 1/cnt constant.

Device strategy (per core, 8-way data-parallel by graph):
  - nodes arrive as 128-row tiles; batch ids are sorted so each 32-graph
    "window" owns a contiguous, per-window padded run of tiles.
  - the whole payload is fp8e4m3: x to ~2^-4 relative (segment-mean averages
    the quantization noise down by ~sqrt(n), n~2000) and clock as hi + lo*512
    pair; measured end-to-end rel err ~2.4e-3 vs the 2e-2 gate.
  - DVE builds one-hot assign tiles [128 nodes, 32 graphs] for a whole
    super-tile in one is_equal op (broadcast AP vs an iota pattern).
  - PE accumulates assign.T @ payload into PSUM [128 graphs, C] fp32 with ONE
    matmul per node-tile. Tiles are interleaved across the 4 windows so
    consecutive matmuls land in different PE column groups (tile_position)
    and overlap in the array.
  - tiny vector-engine epilogue computes the folded per-graph MLP.
"""

import math
import sys
import types

import numpy as np
import ml_dtypes

import concourse.bass as bass
import concourse.bacc as bacc
import concourse.tile as tile
from concourse import mybir
from concourse.bass_utils import run_bass_kernel_spmd


def _ensure_axon_hooks():
    """bass_utils' trace path does `from antenv.axon_hooks import ...`;
    some agent images lack that submodule. Install it (with the real NTFF
    hook when available) so trace=True degrades gracefully instead of
    raising ModuleNotFoundError."""
    try:
        import antenv  # noqa: F401
        import antenv.axon_hooks  # noqa: F401
        return
    except ImportError:
        pass
    try:
        import antenv
    except ImportError:
        return
    mod = types.ModuleType("antenv.axon_hooks")
    state = {"hook": None}
    mod.set_axon_ntff_profile_hook = lambda h: state.__setitem__("hook", h)
    mod.get_axon_ntff_profile_hook = lambda: state["hook"]
    sys.modules["antenv.axon_hooks"] = mod
    antenv.axon_hooks = mod
    try:
        from trn_agent_boot.trn_boot import _ntff_profile_via_ctypes
        mod.set_axon_ntff_profile_hook(
            _ntff_profile_via_ctypes("/opt/axon/libaxon_pjrt.so"))
    except Exception:
        pass
    # the trace path also uploads the NEFF dir to a bucket; in zero-egress
    # containers that raises — fall back to the local path.
    try:
        import concourse.bass_utils as _bu
        _orig_upload = _bu.upload_artifacts

        def _safe_upload(tmpdir):
            try:
                return _orig_upload(tmpdir)
            except Exception:
                return str(tmpdir)

        _bu.upload_artifacts = _safe_upload
    except Exception:
        pass


_ensure_axon_hooks()

BF16 = ml_dtypes.bfloat16
F8 = ml_dtypes.float8_e4m3

N_CORES = 8
N_GRAPHS = 1024
D = 128                 # feature dim of x
GPC = N_GRAPHS // N_CORES   # graphs per core = 128
W = 32                  # one-hot window width (PSUM partition alignment unit)
WPC = GPC // W          # windows per core = 4
ST = 64                 # node-tiles per DMA super-tile (ST % WPC == 0)
LO_SCALE = 512.0        # fp8 lo-correction pre-scale (2^9)


def _build_program(S, C, R):
    """Build the SPMD Bass/Tile program. Shapes are static; per-core data
    differences live entirely in the input tensors.

    S: number of super-tiles (each ST node-tiles of 128 nodes)
    C: fp8 payload column count = 128 + 2*R
    """
    fp32 = mybir.dt.float32
    bf16 = mybir.dt.bfloat16
    f8 = mybir.dt.float8e4
    n_tiles = S * ST

    nc = bacc.Bacc("TRN2", target_bir_lowering=False, debug=False,
                   num_devices=N_CORES)

    # bf16 block: [brall (S*ST) | iota (W)]; fp32 block:
    # [wb (D) | v1 (32) | m2 (R*32) | v0 (32) | w4 (32) | bb | b4 | rec]
    NB = S * ST + W
    NF = D + 32 + R * 32 + 32 + 32 + 3
    xcc = nc.dram_tensor("xcc", [S, 128, ST * C], f8, kind="ExternalInput").ap()
    cb16 = nc.dram_tensor("cb16", [128, NB], bf16, kind="ExternalInput").ap()
    cb32 = nc.dram_tensor("cb32", [128, NF], fp32, kind="ExternalInput").ap()
    # out rides as 4 rows of 32 (from the 32x32 block transpose of the
    # per-graph column) so the final store is 4 single-partition descriptors
    # instead of a 128-partition spray with 16 straggling HBM write receipts
    out_d = nc.dram_tensor("out", [4, 32], fp32, kind="ExternalOutput").ap()

    with tile.TileContext(nc) as tc:
        with (
            tc.tile_pool(name="consts", bufs=1) as cpool,
            tc.tile_pool(name="xin", bufs=8) as xpool,
            tc.tile_pool(name="assign", bufs=1) as apool,
            tc.tile_pool(name="epi", bufs=1) as epool,
            tc.tile_pool(name="ps", bufs=1, space="PSUM") as ppool,
        ):
            # ---- constants: two batched DMAs instead of eleven small ones.
            # cb16 (batch ids) leads the SP ring — the one-hot build needs it
            # first; the fp32 epilogue block rides the ACT ring.
            cb16_t = cpool.tile([128, NB], bf16, tag="cb16")
            nc.sync.dma_start(cb16_t[:], cb16)
            cb32_t = cpool.tile([128, NF], fp32, tag="cb32")
            nc.scalar.dma_start(cb32_t[:], cb32)
            brall = cb16_t[:, 0 : S * ST]
            iota_t = cb16_t[:, S * ST : S * ST + W]
            o = 0
            wb_t = cb32_t[:, o : o + D]; o += D
            v1_t = cb32_t[:, o : o + 32]; o += 32
            m2_t = cb32_t[:, o : o + R * 32]; o += R * 32
            v0_t = cb32_t[:, o : o + 32]; o += 32
            w4_t = cb32_t[:, o : o + 32]; o += 32
            bbt = cb32_t[:, o : o + 1]; o += 1
            b4t = cb32_t[:, o : o + 1]; o += 1
            rect = cb32_t[:, o : o + 1]; o += 1

            psum = ppool.tile([128, C], fp32, tag="acc")

            # init matmul: zero weights x zero rhs, start=True claims the
            # whole bank's has_written bits so all later matmuls (start=False)
            # overwrite-on-first-touch / accumulate-after, independent of
            # window interleaving.
            zw = cpool.tile([128, 128], bf16, tag="zw")
            nc.vector.memset(zw[:], 0.0)
            zr = cpool.tile([128, C], bf16, tag="zr")
            nc.vector.memset(zr[:], 0.0)
            nc.tensor.matmul(psum[:, :], zw[:], zr[:], start=True, stop=False)

            # ---- one-hot assign tiles, built up-front off the critical path:
            # they depend only on constants, so the DVE runs ahead of the
            # DMA/PE pipeline instead of pacing it per super-tile.
            # asg[p, t, j] = (iota[j] == br[p, s*ST + t])
            asgs = []
            for s in range(S):
                asg = apool.tile([128, ST * W], bf16, tag=f"asg{s}")
                nc.vector.tensor_tensor(
                    asg[:].rearrange("p (t j) -> p t j", j=W),
                    iota_t.rearrange("p (o j) -> p o j", o=1)
                        .to_broadcast((128, ST, W)),
                    brall[:, s * ST : (s + 1) * ST]
                        .rearrange("p (t o) -> p t o", o=1)
                        .to_broadcast((128, ST, W)),
                    op=mybir.AluOpType.is_equal,
                )
                asgs.append(asg)

            # ---- main loop ----
            H = (ST // 2) * C
            for s in range(S):
                xt = xpool.tile([128, ST * C], f8, tag="xt")
                # split each super across BOTH HWDGE rings (SP + ACT) so the
                # two rings stream one super concurrently; subtile deps let
                # the first half's matmuls start before the second half lands
                nc.sync.dma_start(xt[:, 0:H], xcc[s][:, 0:H])
                nc.scalar.dma_start(xt[:, H : 2 * H], xcc[s][:, H : 2 * H])
                asg = asgs[s]
                for t in range(ST):
                    i = s * ST + t
                    w = i % WPC         # column-group interleave across windows
                    last = i == n_tiles - 1
                    nc.tensor.matmul(
                        psum[w * W : (w + 1) * W, 0:C],
                        asg[:, t * W : (t + 1) * W],
                        xt[:, t * C : (t + 1) * C],
                        start=False,
                        stop=last,
                        tile_position=(0, w * W),
                    )

            # ---- epilogue (per-graph folded MLP); reads PSUM directly ----
            # Sr = hi_sums + lo_sums / LO_SCALE, then mean via rec
            slo = epool.tile([128, R], fp32, tag="slo")
            nc.vector.tensor_scalar_mul(slo[:], psum[:, D + R : D + 2 * R],
                                        1.0 / LO_SCALE)
            sr = epool.tile([128, R], fp32, tag="sr")
            nc.vector.tensor_add(sr[:], psum[:, D : D + R], slo[:])
            mr = epool.tile([128, R], fp32, tag="mr")
            nc.vector.tensor_scalar_mul(mr[:], sr[:], rect)

            # gp = rowsum(Sx * W_base) * rec + b_base
            t1 = epool.tile([128, D], fp32, tag="t1")
            nc.vector.tensor_mul(t1[:], psum[:, 0:D], wb_t)
            gp = epool.tile([128, 1], fp32, tag="gp")
            nc.vector.tensor_reduce(gp[:], t1[:], axis=mybir.AxisListType.X,
                                    op=mybir.AluOpType.add)
            nc.vector.tensor_scalar(gp[:], gp[:], rect, bbt,
                                    op0=mybir.AluOpType.mult,
                                    op1=mybir.AluOpType.add)

            # pre = gp*v1 + sum_j mr[:,j]*M2[j] + v0
            pre = epool.tile([128, 32], fp32, tag="pre")
            nc.vector.tensor_scalar_mul(pre[:], v1_t, gp[:])
            tmp = epool.tile([128, 32], fp32, tag="tmp")
            for j in range(R):
                nc.vector.tensor_scalar(
                    tmp[:], m2_t[:, j * 32 : (j + 1) * 32], mr[:, j : j + 1], None,
                    op0=mybir.AluOpType.mult,
                )
                nc.vector.tensor_add(pre[:], pre[:], tmp[:])
            nc.vector.tensor_add(pre[:], pre[:], v0_t)

            act = epool.tile([128, 32], fp32, tag="act")
            nc.vector.tensor_scalar_max(act[:], pre[:], 0.0)

            # out = rowsum(act * W4) + b4
            nc.vector.tensor_mul(act[:], act[:], w4_t)
            oo = epool.tile([128, 32], fp32, tag="oo")
            nc.vector.memset(oo[:], 0.0)
            nc.vector.tensor_reduce(oo[:, 0:1], act[:], axis=mybir.AxisListType.X,
                                    op=mybir.AluOpType.add)
            nc.vector.tensor_add(oo[:, 0:1], oo[:, 0:1], b4t)

            # 32x32 block transpose: row 32*a of oot holds graphs 32a..32a+31,
            # so the store is 4 contiguous single-partition rows (2 per ring)
            oot = epool.tile([128, 32], fp32, tag="oot")
            nc.vector.transpose(oot[:], oo[:])
            for a in range(4):
                eng = nc.sync if a % 2 == 0 else nc.scalar
                eng.dma_start(out_d[a : a + 1, :], oot[32 * a : 32 * a + 1, :])

    nc.compile()
    return nc


def kernel(x, clock_period, batch, W_base, b_base, W1, b1, W2, b2, W3, b3, W4, b4,
           _profile=None):
    x = np.asarray(x, np.float32)
    clock = np.asarray(clock_period, np.float32).reshape(-1)
    batch = np.asarray(batch, np.int32)
    W_base = np.asarray(W_base, np.float32)
    W1 = np.asarray(W1, np.float32); b1 = np.asarray(b1, np.float32)
    W2 = np.asarray(W2, np.float32); b2 = np.asarray(b2, np.float32)
    W3 = np.asarray(W3, np.float32); b3 = np.asarray(b3, np.float32)
    W4 = np.asarray(W4, np.float32); b4 = np.asarray(b4, np.float32)
    hid = W1.shape[1]

    # r-path: exact algebraic fold when relu(c*W1 + b1) == c * relu(W1)
    fold = bool(np.all(b1 == 0.0)) and bool(clock.min() >= 0.0)
    if fold:
        R = 1
        r32 = clock[:, None]                                   # [N, 1]
        q = np.maximum(W1, 0.0) @ W2                           # [1, hid]
        M2 = q @ W3[1:, :]                                     # [1, 32]
        v0 = b2 @ W3[1:, :] + b3                               # [32]
    else:
        R = hid
        r32 = np.maximum(clock[:, None] @ W1 + b1, 0.0)        # [N, hid]
        M2 = W2 @ W3[1:, :]                                    # [hid, 32]
        v0 = b2 @ W3[1:, :] + b3

    C = D + 2 * R           # [x | r_hi | r_lo], all fp8e4m3

    # ---- shard by graph; window padding so tile->window map is static ----
    cut = np.searchsorted(batch, np.arange(0, N_GRAPHS + 1, W))
    T_w = int(math.ceil(np.diff(cut).max() / 128.0))
    tpw = ST // WPC         # tiles of one window inside one super-tile
    while T_w % tpw:
        T_w += 1
    n_tiles = WPC * T_w
    S = n_tiles // ST

    gcut = np.searchsorted(batch, np.arange(0, N_GRAPHS + 1))
    cnt = np.diff(gcut).astype(np.float32)
    rec_all = (1.0 / np.maximum(cnt, 1.0)).astype(np.float32)

    x8 = x.astype(F8)
    rhi = r32.astype(F8)
    rlo = ((r32 - rhi.astype(np.float32)) * LO_SCALE).astype(F8)

    in_maps = []
    # shared constant blocks
    iota_c = np.arange(W, dtype=np.float32)
    cb32_shared = np.concatenate([
        W_base[:, 0], W3[0, :], M2.reshape(-1), v0, W4[:, 0],
        [float(b_base.reshape(-1)[0])], [float(b4.reshape(-1)[0])],
    ]).astype(np.float32)

    for k in range(N_CORES):
        wx = np.zeros((WPC, T_w * 128, C), F8)
        wbr = np.full((WPC, T_w * 128), -1.0, BF16)
        for wi in range(WPC):
            gw = k * WPC + wi          # global window index
            s0, e0 = int(cut[gw]), int(cut[gw + 1])
            n = e0 - s0
            wx[wi, :n, 0:D] = x8[s0:e0]
            wx[wi, :n, D : D + R] = rhi[s0:e0]
            wx[wi, :n, D + R : D + 2 * R] = rlo[s0:e0]
            wbr[wi, :n] = (batch[s0:e0] - gw * W).astype(BF16)
        # window-interleaved tile order: tile i = s*ST + t belongs to window
        # i % WPC at within-window slot i // WPC; each SBUF partition line is
        # contiguous in DRAM.
        xcc_p = np.ascontiguousarray(
            wx.reshape(WPC, S, tpw, 128, C).transpose(1, 3, 2, 0, 4)
        ).reshape(S, 128, ST * C)
        brs_p = np.ascontiguousarray(
            wbr.reshape(WPC, S, tpw, 128).transpose(3, 1, 2, 0)
        ).reshape(128, S * ST)
        cb16_k = np.concatenate(
            [brs_p.astype(BF16),
             np.broadcast_to(iota_c[None, :], (128, W)).astype(BF16)], axis=1)
        rec_b = rec_all[k * GPC : (k + 1) * GPC]
        cb32_k = np.concatenate([
            np.broadcast_to(cb32_shared[None, :], (128, len(cb32_shared))),
            rec_b.reshape(128, 1),
        ], axis=1).astype(np.float32)
        in_maps.append(dict(xcc=xcc_p, cb16=np.ascontiguousarray(cb16_k),
                            cb32=np.ascontiguousarray(cb32_k)))

    nc = _build_program(S, C, R)

    kw = {}
    if _profile is not None:
        kw = dict(trace=True, **_profile)
    res = run_bass_kernel_spmd(nc, in_maps, list(range(N_CORES)), **kw)

    out = np.concatenate(
        [res.results[k]["out"].reshape(GPC, 1) for k in range(N_CORES)], axis=0)
    if _profile is not None:
        return out.astype(np.float32), res
    return out.astype(np.float32)


# revision 32
# speedup vs baseline: 1.0303x; 1.0303x over previous
"""Trainium2 Bass kernel for nn_ClockAwareGNN (segment_reduce).

Model (reference, fp32):
    gp   = segment_mean(x, batch) @ W_base + b_base            # [B, 1]
    h    = relu(clock @ W1 + b1) @ W2 + b2                     # [N, 16]
    cp   = segment_mean(h, batch)                              # [B, 16]
    out  = relu([gp | cp] @ W3 + b3) @ W4 + b4                 # [B, 1]

Everything after the segment reductions is affine in per-graph quantities, so
the heavy per-node work collapses to fused segment sums:
    Sx[g] = sum of x rows in graph g           (128 cols, fp8 payload)
    Sr[g] = sum of r rows in graph g           (R cols, fp8 hi + fp8 lo*512)
where r is either the raw clock (R=1; exact when b1 == 0 and clock >= 0 since
relu(c*W1) == c*relu(W1) elementwise for c >= 0) or the host-computed
relu(clock @ W1 + b1) (R=16 fallback). Graph node counts come from `batch` on
the host (they are index metadata), shipped as a per-graph 1/cnt constant.

Device strategy (per core, 8-way data-parallel by graph):
  - nodes arrive as 128-row tiles; batch ids are sorted so each 32-graph
    "window" owns a contiguous, per-window padded run of tiles.
  - the whole payload is fp8e4m3: x to ~2^-4 relative (segment-mean averages
    the quantization noise down by ~sqrt(n), n~2000) and clock as hi + lo*512
    pair; measured end-to-end rel err ~2.4e-3 vs the 2e-2 gate.
  - DVE builds one-hot assign tiles [128 nodes, 32 graphs] for a whole
    super-tile in one is_equal op (broadcast AP vs an iota pattern).
  - PE accumulates assign.T @ payload into PSUM [128 graphs, C] fp32 with ONE
    matmul per node-tile. Tiles are interleaved across the 4 windows so
    consecutive matmuls land in different PE column groups (tile_position)
    and overlap in the array.
  - tiny vector-engine epilogue computes the folded per-graph MLP.
"""

import math
import sys
import types

import numpy as np
import ml_dtypes

import concourse.bass as bass
import concourse.bacc as bacc
import concourse.tile as tile
from concourse import mybir
from concourse.bass_utils import run_bass_kernel_spmd


def _ensure_axon_hooks():
    """bass_utils' trace path does `from antenv.axon_hooks import ...`;
    some agent images lack that submodule. Install it (with the real NTFF
    hook when available) so trace=True degrades gracefully instead of
    raising ModuleNotFoundError."""
    try:
        import antenv  # noqa: F401
        import antenv.axon_hooks  # noqa: F401
        return
    except ImportError:
        pass
    try:
        import antenv
    except ImportError:
        return
    mod = types.ModuleType("antenv.axon_hooks")
    state = {"hook": None}
    mod.set_axon_ntff_profile_hook = lambda h: state.__setitem__("hook", h)
    mod.get_axon_ntff_profile_hook = lambda: state["hook"]
    sys.modules["antenv.axon_hooks"] = mod
    antenv.axon_hooks = mod
    try:
        from trn_agent_boot.trn_boot import _ntff_profile_via_ctypes
        mod.set_axon_ntff_profile_hook(
            _ntff_profile_via_ctypes("/opt/axon/libaxon_pjrt.so"))
    except Exception:
        pass
    # the trace path also uploads the NEFF dir to a bucket; in zero-egress
    # containers that raises — fall back to the local path.
    try:
        import concourse.bass_utils as _bu
        _orig_upload = _bu.upload_artifacts

        def _safe_upload(tmpdir):
            try:
                return _orig_upload(tmpdir)
            except Exception:
                return str(tmpdir)

        _bu.upload_artifacts = _safe_upload
    except Exception:
        pass


_ensure_axon_hooks()

BF16 = ml_dtypes.bfloat16
F8 = ml_dtypes.float8_e4m3

N_CORES = 8
N_GRAPHS = 1024
D = 128                 # feature dim of x
GPC = N_GRAPHS // N_CORES   # graphs per core = 128
W = 32                  # one-hot window width (PSUM partition alignment unit)
WPC = GPC // W          # windows per core = 4
ST = 64                 # node-tiles per DMA super-tile (ST % WPC == 0)
LO_SCALE = 512.0        # fp8 lo-correction pre-scale (2^9)


def _build_program(S, C, R):
    """Build the SPMD Bass/Tile program. Shapes are static; per-core data
    differences live entirely in the input tensors.

    S: number of super-tiles (each ST node-tiles of 128 nodes)
    C: fp8 payload column count = 128 + 2*R
    """
    fp32 = mybir.dt.float32
    bf16 = mybir.dt.bfloat16
    f8 = mybir.dt.float8e4
    u8 = mybir.dt.uint8
    n_tiles = S * ST

    nc = bacc.Bacc("TRN2", target_bir_lowering=False, debug=False,
                   num_devices=N_CORES)

    # u8 block: [brall (S*ST) | iota (W)]; fp32 block:
    # [wb (D) | v1 (32) | m2 (R*32) | v0 (32) | w4 (32) | bb | b4 | rec |
    #  pid (1, =partition index) | iot128 (128, 0..127 row)]
    NB = S * ST + W
    NF = D + 32 + R * 32 + 32 + 32 + 3 + 1 + 128
    xcc = nc.dram_tensor("xcc", [S, 128, ST * C], f8, kind="ExternalInput").ap()
    cb16 = nc.dram_tensor("cb16", [128, NB], u8, kind="ExternalInput").ap()
    cb32 = nc.dram_tensor("cb32", [128, NF], fp32, kind="ExternalInput").ap()
    # out is one [1,128] row (PE-transposed) so the final store is a single
    # contiguous descriptor with one HBM write receipt instead of a
    # 128-partition spray with 16 straggling ones
    out_d = nc.dram_tensor("out", [1, 128], fp32, kind="ExternalOutput").ap()

    with tile.TileContext(nc) as tc:
        with (
            tc.tile_pool(name="consts", bufs=1) as cpool,
            tc.tile_pool(name="xin", bufs=8) as xpool,
            tc.tile_pool(name="assign", bufs=1) as apool,
            tc.tile_pool(name="epi", bufs=1) as epool,
            tc.tile_pool(name="ps", bufs=1, space="PSUM") as ppool,
        ):
            # ---- constants: two batched DMAs instead of eleven small ones.
            # cb16 (batch ids) leads the SP ring — the one-hot build needs it
            # first; the fp32 epilogue block rides the ACT ring.
            cb16_t = cpool.tile([128, NB], u8, tag="cb16")
            nc.sync.dma_start(cb16_t[:], cb16)
            cb32_t = cpool.tile([128, NF], fp32, tag="cb32")
            nc.scalar.dma_start(cb32_t[:], cb32)
            brall = cb16_t[:, 0 : S * ST]
            iota_t = cb16_t[:, S * ST : S * ST + W]
            o = 0
            wb_t = cb32_t[:, o : o + D]; o += D
            v1_t = cb32_t[:, o : o + 32]; o += 32
            m2_t = cb32_t[:, o : o + R * 32]; o += R * 32
            v0_t = cb32_t[:, o : o + 32]; o += 32
            w4_t = cb32_t[:, o : o + 32]; o += 32
            bbt = cb32_t[:, o : o + 1]; o += 1
            b4t = cb32_t[:, o : o + 1]; o += 1
            rect = cb32_t[:, o : o + 1]; o += 1
            pid_t = cb32_t[:, o : o + 1]; o += 1
            iot_t = cb32_t[:, o : o + 128]; o += 128

            psum = ppool.tile([128, C], fp32, tag="acc")

            # identity weights (doubles as the final PE-transpose operand);
            # the init matmul I.T @ 0 still writes zeros with start=True,
            # claiming the whole bank's has_written bits so all later matmuls
            # (start=False) overwrite-on-first-touch / accumulate-after,
            # independent of window interleaving.
            zw = cpool.tile([128, 128], fp32, tag="zw")
            nc.vector.tensor_tensor(
                zw[:], iot_t,
                pid_t.to_broadcast((128, 128)),
                op=mybir.AluOpType.is_equal,
            )
            zr = cpool.tile([128, C], fp32, tag="zr")
            nc.vector.memset(zr[:], 0.0)
            nc.tensor.matmul(psum[:, :], zw[:], zr[:], start=True, stop=False)

            # ---- one-hot assign tiles, built up-front off the critical path:
            # they depend only on constants, so the DVE runs ahead of the
            # DMA/PE pipeline instead of pacing it per super-tile.
            # asg[p, t, j] = (iota[j] == br[p, s*ST + t])
            asgs = []
            for s in range(S):
                asg = apool.tile([128, ST * W], bf16, tag=f"asg{s}")
                nc.vector.tensor_tensor(
                    asg[:].rearrange("p (t j) -> p t j", j=W),
                    iota_t.rearrange("p (o j) -> p o j", o=1)
                        .to_broadcast((128, ST, W)),
                    brall[:, s * ST : (s + 1) * ST]
                        .rearrange("p (t o) -> p t o", o=1)
                        .to_broadcast((128, ST, W)),
                    op=mybir.AluOpType.is_equal,
                )
                asgs.append(asg)

            # ---- main loop ----
            H = (ST // 2) * C
            for s in range(S):
                xt = xpool.tile([128, ST * C], f8, tag="xt")
                # split each super across BOTH HWDGE rings (SP + ACT) so the
                # two rings stream one super concurrently; subtile deps let
                # the first half's matmuls start before the second half lands
                nc.sync.dma_start(xt[:, 0:H], xcc[s][:, 0:H])
                nc.scalar.dma_start(xt[:, H : 2 * H], xcc[s][:, H : 2 * H])
                asg = asgs[s]
                for t in range(ST):
                    i = s * ST + t
                    w = i % WPC         # column-group interleave across windows
                    last = i == n_tiles - 1
                    nc.tensor.matmul(
                        psum[w * W : (w + 1) * W, 0:C],
                        asg[:, t * W : (t + 1) * W],
                        xt[:, t * C : (t + 1) * C],
                        start=False,
                        stop=last,
                        tile_position=(0, w * W),
                    )

            # ---- epilogue (per-graph folded MLP); reads PSUM directly ----
            mr = epool.tile([128, R], fp32, tag="mr")
            nc.vector.tensor_scalar_mul(mr[:], psum[:, D : D + R], rect)

            # gp = rowsum(Sx * W_base) * rec + b_base
            t1 = epool.tile([128, D], fp32, tag="t1")
            nc.vector.tensor_mul(t1[:], psum[:, 0:D], wb_t)
            gp = epool.tile([128, 1], fp32, tag="gp")
            nc.vector.tensor_reduce(gp[:], t1[:], axis=mybir.AxisListType.X,
                                    op=mybir.AluOpType.add)
            nc.vector.tensor_scalar(gp[:], gp[:], rect, bbt,
                                    op0=mybir.AluOpType.mult,
                                    op1=mybir.AluOpType.add)

            # pre = gp*v1 + sum_j mr[:,j]*M2[j] + v0
            pre = epool.tile([128, 32], fp32, tag="pre")
            nc.vector.tensor_scalar_mul(pre[:], v1_t, gp[:])
            tmp = epool.tile([128, 32], fp32, tag="tmp")
            for j in range(R):
                nc.vector.tensor_scalar(
                    tmp[:], m2_t[:, j * 32 : (j + 1) * 32], mr[:, j : j + 1], None,
                    op0=mybir.AluOpType.mult,
                )
                nc.vector.tensor_add(pre[:], pre[:], tmp[:])
            nc.vector.tensor_add(pre[:], pre[:], v0_t)

            act = epool.tile([128, 32], fp32, tag="act")
            nc.vector.tensor_scalar_max(act[:], pre[:], 0.0)

            # out = rowsum(act * W4) + b4
            nc.vector.tensor_mul(act[:], act[:], w4_t)
            oo = epool.tile([128, 1], fp32, tag="oo")
            nc.vector.tensor_reduce(oo[:], act[:], axis=mybir.AxisListType.X,
                                    op=mybir.AluOpType.add)
            nc.vector.tensor_add(oo[:], oo[:], b4t)

            # PE transpose (oo.T @ I) lands all 128 graph outputs in ONE psum
            # partition row -> one contiguous 512B store, one write receipt
            ps2 = ppool.tile([1, 128], fp32, tag="acc2")
            nc.tensor.transpose(ps2[:], oo[:], zw[:])
            orow = epool.tile([1, 128], fp32, tag="orow")
            nc.vector.tensor_copy(orow[:], ps2[:])
            nc.sync.dma_start(out_d, orow[:])

    nc.compile()
    return nc


def kernel(x, clock_period, batch, W_base, b_base, W1, b1, W2, b2, W3, b3, W4, b4,
           _profile=None):
    x = np.asarray(x, np.float32)
    clock = np.asarray(clock_period, np.float32).reshape(-1)
    batch = np.asarray(batch, np.int32)
    W_base = np.asarray(W_base, np.float32)
    W1 = np.asarray(W1, np.float32); b1 = np.asarray(b1, np.float32)
    W2 = np.asarray(W2, np.float32); b2 = np.asarray(b2, np.float32)
    W3 = np.asarray(W3, np.float32); b3 = np.asarray(b3, np.float32)
    W4 = np.asarray(W4, np.float32); b4 = np.asarray(b4, np.float32)
    hid = W1.shape[1]

    # r-path: exact algebraic fold when relu(c*W1 + b1) == c * relu(W1)
    fold = bool(np.all(b1 == 0.0)) and bool(clock.min() >= 0.0)
    if fold:
        R = 1
        r32 = clock[:, None]                                   # [N, 1]
        q = np.maximum(W1, 0.0) @ W2                           # [1, hid]
        M2 = q @ W3[1:, :]                                     # [1, 32]
        v0 = b2 @ W3[1:, :] + b3                               # [32]
    else:
        R = hid
        r32 = np.maximum(clock[:, None] @ W1 + b1, 0.0)        # [N, hid]
        M2 = W2 @ W3[1:, :]                                    # [hid, 32]
        v0 = b2 @ W3[1:, :] + b3

    C = D + R               # [x | r], all fp8e4m3

    # ---- shard by graph; window padding so tile->window map is static ----
    cut = np.searchsorted(batch, np.arange(0, N_GRAPHS + 1, W))
    T_w = int(math.ceil(np.diff(cut).max() / 128.0))
    tpw = ST // WPC         # tiles of one window inside one super-tile
    while T_w % tpw:
        T_w += 1
    n_tiles = WPC * T_w
    S = n_tiles // ST

    gcut = np.searchsorted(batch, np.arange(0, N_GRAPHS + 1))
    cnt = np.diff(gcut).astype(np.float32)
    rec_all = (1.0 / np.maximum(cnt, 1.0)).astype(np.float32)

    x8 = x.astype(F8)
    rhi = r32.astype(F8)

    in_maps = []
    # shared constant blocks
    cb32_shared = np.concatenate([
        W_base[:, 0], W3[0, :], M2.reshape(-1), v0, W4[:, 0],
        [float(b_base.reshape(-1)[0])], [float(b4.reshape(-1)[0])],
    ]).astype(np.float32)
    pid_iot = np.concatenate([
        np.arange(128, dtype=np.float32).reshape(128, 1),       # pid
        np.broadcast_to(np.arange(128, dtype=np.float32)[None, :], (128, 128)),
    ], axis=1)

    for k in range(N_CORES):
        wx = np.zeros((WPC, T_w * 128, C), F8)
        wbr = np.full((WPC, T_w * 128), 255, np.uint8)
        for wi in range(WPC):
            gw = k * WPC + wi          # global window index
            s0, e0 = int(cut[gw]), int(cut[gw + 1])
            n = e0 - s0
            wx[wi, :n, 0:D] = x8[s0:e0]
            wx[wi, :n, D : D + R] = rhi[s0:e0]
            wbr[wi, :n] = (batch[s0:e0] - gw * W).astype(np.uint8)
        # window-interleaved tile order: tile i = s*ST + t belongs to window
        # i % WPC at within-window slot i // WPC; each SBUF partition line is
        # contiguous in DRAM.
        xcc_p = np.ascontiguousarray(
            wx.reshape(WPC, S, tpw, 128, C).transpose(1, 3, 2, 0, 4)
        ).reshape(S, 128, ST * C)
        brs_p = np.ascontiguousarray(
            wbr.reshape(WPC, S, tpw, 128).transpose(3, 1, 2, 0)
        ).reshape(128, S * ST)
        cb16_k = np.concatenate(
            [brs_p,
             np.broadcast_to(np.arange(W, dtype=np.uint8)[None, :], (128, W))],
            axis=1)
        rec_b = rec_all[k * GPC : (k + 1) * GPC]
        cb32_k = np.concatenate([
            np.broadcast_to(cb32_shared[None, :], (128, len(cb32_shared))),
            rec_b.reshape(128, 1),
            pid_iot,
        ], axis=1).astype(np.float32)
        in_maps.append(dict(xcc=xcc_p, cb16=np.ascontiguousarray(cb16_k),
                            cb32=np.ascontiguousarray(cb32_k)))

    nc = _build_program(S, C, R)

    kw = {}
    if _profile is not None:
        kw = dict(trace=True, **_profile)
    res = run_bass_kernel_spmd(nc, in_maps, list(range(N_CORES)), **kw)

    out = np.concatenate(
        [res.results[k]["out"].reshape(GPC, 1) for k in range(N_CORES)], axis=0)
    if _profile is not None:
        return out.astype(np.float32), res
    return out.astype(np.float32)


# revision 33
# speedup vs baseline: 1.0342x; 1.0039x over previous
"""Trainium2 Bass kernel for nn_ClockAwareGNN (segment_reduce).

Model (reference, fp32):
    gp   = segment_mean(x, batch) @ W_base + b_base            # [B, 1]
    h    = relu(clock @ W1 + b1) @ W2 + b2                     # [N, 16]
    cp   = segment_mean(h, batch)                              # [B, 16]
    out  = relu([gp | cp] @ W3 + b3) @ W4 + b4                 # [B, 1]

Everything after the segment reductions is affine in per-graph quantities, so
the heavy per-node work collapses to fused segment sums:
    Sx[g] = sum of x rows in graph g           (128 cols, fp8 payload)
    Sr[g] = sum of r rows in graph g           (R cols, fp8 hi + fp8 lo*512)
where r is either the raw clock (R=1; exact when b1 == 0 and clock >= 0 since
relu(c*W1) == c*relu(W1) elementwise for c >= 0) or the host-computed
relu(clock @ W1 + b1) (R=16 fallback). Graph node counts come from `batch` on
the host (they are index metadata), shipped as a per-graph 1/cnt constant.

Device strategy (per core, 8-way data-parallel by graph):
  - nodes arrive as 128-row tiles; batch ids are sorted so each 32-graph
    "window" owns a contiguous, per-window padded run of tiles.
  - the whole payload is fp8e4m3: x to ~2^-4 relative (segment-mean averages
    the quantization noise down by ~sqrt(n), n~2000) and clock as hi + lo*512
    pair; measured end-to-end rel err ~2.4e-3 vs the 2e-2 gate.
  - DVE builds one-hot assign tiles [128 nodes, 32 graphs] for a whole
    super-tile in one is_equal op (broadcast AP vs an iota pattern).
  - PE accumulates assign.T @ payload into PSUM [128 graphs, C] fp32 with ONE
    matmul per node-tile. Tiles are interleaved across the 4 windows so
    consecutive matmuls land in different PE column groups (tile_position)
    and overlap in the array.
  - tiny vector-engine epilogue computes the folded per-graph MLP.
"""

import math
import sys
import types

import numpy as np
import ml_dtypes

import concourse.bass as bass
import concourse.bacc as bacc
import concourse.tile as tile
from concourse import mybir
from concourse.bass_utils import run_bass_kernel_spmd


def _ensure_axon_hooks():
    """bass_utils' trace path does `from antenv.axon_hooks import ...`;
    some agent images lack that submodule. Install it (with the real NTFF
    hook when available) so trace=True degrades gracefully instead of
    raising ModuleNotFoundError."""
    try:
        import antenv  # noqa: F401
        import antenv.axon_hooks  # noqa: F401
        return
    except ImportError:
        pass
    try:
        import antenv
    except ImportError:
        return
    mod = types.ModuleType("antenv.axon_hooks")
    state = {"hook": None}
    mod.set_axon_ntff_profile_hook = lambda h: state.__setitem__("hook", h)
    mod.get_axon_ntff_profile_hook = lambda: state["hook"]
    sys.modules["antenv.axon_hooks"] = mod
    antenv.axon_hooks = mod
    try:
        from trn_agent_boot.trn_boot import _ntff_profile_via_ctypes
        mod.set_axon_ntff_profile_hook(
            _ntff_profile_via_ctypes("/opt/axon/libaxon_pjrt.so"))
    except Exception:
        pass
    # the trace path also uploads the NEFF dir to a bucket; in zero-egress
    # containers that raises — fall back to the local path.
    try:
        import concourse.bass_utils as _bu
        _orig_upload = _bu.upload_artifacts

        def _safe_upload(tmpdir):
            try:
                return _orig_upload(tmpdir)
            except Exception:
                return str(tmpdir)

        _bu.upload_artifacts = _safe_upload
    except Exception:
        pass


_ensure_axon_hooks()

BF16 = ml_dtypes.bfloat16
F8 = ml_dtypes.float8_e4m3

N_CORES = 8
N_GRAPHS = 1024
D = 128                 # feature dim of x
GPC = N_GRAPHS // N_CORES   # graphs per core = 128
W = 32                  # one-hot window width (PSUM partition alignment unit)
WPC = GPC // W          # windows per core = 4
ST = 60                 # node-tiles per DMA super-tile (ST % WPC == 0)


def _build_program(S, C, R):
    """Build the SPMD Bass/Tile program. Shapes are static; per-core data
    differences live entirely in the input tensors.

    S: number of super-tiles (each ST node-tiles of 128 nodes)
    C: fp8 payload column count = 128 + 2*R
    """
    fp32 = mybir.dt.float32
    bf16 = mybir.dt.bfloat16
    f8 = mybir.dt.float8e4
    u8 = mybir.dt.uint8
    n_tiles = S * ST

    nc = bacc.Bacc("TRN2", target_bir_lowering=False, debug=False,
                   num_devices=N_CORES)

    # u8 block: [brall (S*ST) | iota (W)]; fp32 block:
    # [wb (D) | v1 (32) | m2 (R*32) | v0 (32) | w4 (32) | bb | b4 | rec |
    #  pid (1, =partition index) | iot128 (128, 0..127 row)]
    NB = S * ST + W
    NF = D + 32 + R * 32 + 32 + 32 + 3 + 1 + 128
    xcc = nc.dram_tensor("xcc", [S, 128, ST * C], f8, kind="ExternalInput").ap()
    cb16 = nc.dram_tensor("cb16", [128, NB], u8, kind="ExternalInput").ap()
    cb32 = nc.dram_tensor("cb32", [128, NF], fp32, kind="ExternalInput").ap()
    # out is one [1,128] row (PE-transposed) so the final store is a single
    # contiguous descriptor with one HBM write receipt instead of a
    # 128-partition spray with 16 straggling ones
    out_d = nc.dram_tensor("out", [1, 128], fp32, kind="ExternalOutput").ap()

    with tile.TileContext(nc) as tc:
        with (
            tc.tile_pool(name="consts", bufs=1) as cpool,
            tc.tile_pool(name="xin", bufs=8) as xpool,
            tc.tile_pool(name="assign", bufs=1) as apool,
            tc.tile_pool(name="epi", bufs=1) as epool,
            tc.tile_pool(name="ps", bufs=1, space="PSUM") as ppool,
        ):
            # ---- constants: two batched DMAs instead of eleven small ones.
            # cb16 (batch ids) leads the SP ring — the one-hot build needs it
            # first; the fp32 epilogue block rides the ACT ring.
            cb16_t = cpool.tile([128, NB], u8, tag="cb16")
            nc.sync.dma_start(cb16_t[:], cb16)
            cb32_t = cpool.tile([128, NF], fp32, tag="cb32")
            nc.scalar.dma_start(cb32_t[:], cb32)
            brall = cb16_t[:, 0 : S * ST]
            iota_t = cb16_t[:, S * ST : S * ST + W]
            o = 0
            wb_t = cb32_t[:, o : o + D]; o += D
            v1_t = cb32_t[:, o : o + 32]; o += 32
            m2_t = cb32_t[:, o : o + R * 32]; o += R * 32
            v0_t = cb32_t[:, o : o + 32]; o += 32
            w4_t = cb32_t[:, o : o + 32]; o += 32
            bbt = cb32_t[:, o : o + 1]; o += 1
            b4t = cb32_t[:, o : o + 1]; o += 1
            rect = cb32_t[:, o : o + 1]; o += 1
            pid_t = cb32_t[:, o : o + 1]; o += 1
            iot_t = cb32_t[:, o : o + 128]; o += 128

            psum = ppool.tile([128, C], fp32, tag="acc")

            # identity weights (doubles as the final PE-transpose operand);
            # the init matmul I.T @ 0 still writes zeros with start=True,
            # claiming the whole bank's has_written bits so all later matmuls
            # (start=False) overwrite-on-first-touch / accumulate-after,
            # independent of window interleaving.
            zw = cpool.tile([128, 128], fp32, tag="zw")
            nc.vector.tensor_tensor(
                zw[:], iot_t,
                pid_t.to_broadcast((128, 128)),
                op=mybir.AluOpType.is_equal,
            )
            zr = cpool.tile([128, C], fp32, tag="zr")
            nc.vector.memset(zr[:], 0.0)
            nc.tensor.matmul(psum[:, :], zw[:], zr[:], start=True, stop=False)

            # ---- one-hot assign tiles, built up-front off the critical path:
            # they depend only on constants, so the DVE runs ahead of the
            # DMA/PE pipeline instead of pacing it per super-tile.
            # asg[p, t, j] = (iota[j] == br[p, s*ST + t])
            asgs = []
            for s in range(S):
                asg = apool.tile([128, ST * W], bf16, tag=f"asg{s}")
                nc.vector.tensor_tensor(
                    asg[:].rearrange("p (t j) -> p t j", j=W),
                    iota_t.rearrange("p (o j) -> p o j", o=1)
                        .to_broadcast((128, ST, W)),
                    brall[:, s * ST : (s + 1) * ST]
                        .rearrange("p (t o) -> p t o", o=1)
                        .to_broadcast((128, ST, W)),
                    op=mybir.AluOpType.is_equal,
                )
                asgs.append(asg)

            # ---- main loop ----
            H = (ST // 2) * C
            for s in range(S):
                xt = xpool.tile([128, ST * C], f8, tag="xt")
                # split each super across BOTH HWDGE rings (SP + ACT) so the
                # two rings stream one super concurrently; subtile deps let
                # the first half's matmuls start before the second half lands
                nc.sync.dma_start(xt[:, 0:H], xcc[s][:, 0:H])
                nc.scalar.dma_start(xt[:, H : 2 * H], xcc[s][:, H : 2 * H])
                asg = asgs[s]
                for t in range(ST):
                    i = s * ST + t
                    w = i % WPC         # column-group interleave across windows
                    last = i == n_tiles - 1
                    nc.tensor.matmul(
                        psum[w * W : (w + 1) * W, 0:C],
                        asg[:, t * W : (t + 1) * W],
                        xt[:, t * C : (t + 1) * C],
                        start=False,
                        stop=last,
                        tile_position=(0, w * W),
                    )

            # ---- epilogue (per-graph folded MLP); reads PSUM directly ----
            mr = epool.tile([128, R], fp32, tag="mr")
            nc.vector.tensor_scalar_mul(mr[:], psum[:, D : D + R], rect)

            # gp = rowsum(Sx * W_base) * rec + b_base
            t1 = epool.tile([128, D], fp32, tag="t1")
            nc.vector.tensor_mul(t1[:], psum[:, 0:D], wb_t)
            gp = epool.tile([128, 1], fp32, tag="gp")
            nc.vector.tensor_reduce(gp[:], t1[:], axis=mybir.AxisListType.X,
                                    op=mybir.AluOpType.add)
            nc.vector.tensor_scalar(gp[:], gp[:], rect, bbt,
                                    op0=mybir.AluOpType.mult,
                                    op1=mybir.AluOpType.add)

            # pre = gp*v1 + sum_j mr[:,j]*M2[j] + v0
            pre = epool.tile([128, 32], fp32, tag="pre")
            nc.vector.tensor_scalar_mul(pre[:], v1_t, gp[:])
            tmp = epool.tile([128, 32], fp32, tag="tmp")
            for j in range(R):
                nc.vector.tensor_scalar(
                    tmp[:], m2_t[:, j * 32 : (j + 1) * 32], mr[:, j : j + 1], None,
                    op0=mybir.AluOpType.mult,
                )
                nc.vector.tensor_add(pre[:], pre[:], tmp[:])
            nc.vector.tensor_add(pre[:], pre[:], v0_t)

            act = epool.tile([128, 32], fp32, tag="act")
            nc.vector.tensor_scalar_max(act[:], pre[:], 0.0)

            # out = rowsum(act * W4) + b4
            nc.vector.tensor_mul(act[:], act[:], w4_t)
            oo = epool.tile([128, 1], fp32, tag="oo")
            nc.vector.tensor_reduce(oo[:], act[:], axis=mybir.AxisListType.X,
                                    op=mybir.AluOpType.add)
            nc.vector.tensor_add(oo[:], oo[:], b4t)

            # PE transpose (oo.T @ I) lands all 128 graph outputs in ONE psum
            # partition row -> one contiguous 512B store, one write receipt
            ps2 = ppool.tile([1, 128], fp32, tag="acc2")
            nc.tensor.transpose(ps2[:], oo[:], zw[:])
            orow = epool.tile([1, 128], fp32, tag="orow")
            nc.vector.tensor_copy(orow[:], ps2[:])
            nc.sync.dma_start(out_d, orow[:])

    nc.compile()
    return nc


def kernel(x, clock_period, batch, W_base, b_base, W1, b1, W2, b2, W3, b3, W4, b4,
           _profile=None):
    x = np.asarray(x, np.float32)
    clock = np.asarray(clock_period, np.float32).reshape(-1)
    batch = np.asarray(batch, np.int32)
    W_base = np.asarray(W_base, np.float32)
    W1 = np.asarray(W1, np.float32); b1 = np.asarray(b1, np.float32)
    W2 = np.asarray(W2, np.float32); b2 = np.asarray(b2, np.float32)
    W3 = np.asarray(W3, np.float32); b3 = np.asarray(b3, np.float32)
    W4 = np.asarray(W4, np.float32); b4 = np.asarray(b4, np.float32)
    hid = W1.shape[1]

    # r-path: exact algebraic fold when relu(c*W1 + b1) == c * relu(W1)
    fold = bool(np.all(b1 == 0.0)) and bool(clock.min() >= 0.0)
    if fold:
        R = 1
        r32 = clock[:, None]                                   # [N, 1]
        q = np.maximum(W1, 0.0) @ W2                           # [1, hid]
        M2 = q @ W3[1:, :]                                     # [1, 32]
        v0 = b2 @ W3[1:, :] + b3                               # [32]
    else:
        R = hid
        r32 = np.maximum(clock[:, None] @ W1 + b1, 0.0)        # [N, hid]
        M2 = W2 @ W3[1:, :]                                    # [hid, 32]
        v0 = b2 @ W3[1:, :] + b3

    C = D + R               # [x | r], all fp8e4m3

    # ---- shard by graph; window padding so tile->window map is static ----
    cut = np.searchsorted(batch, np.arange(0, N_GRAPHS + 1, W))
    T_w = int(math.ceil(np.diff(cut).max() / 128.0))
    tpw = ST // WPC         # tiles of one window inside one super-tile
    while T_w % tpw:
        T_w += 1
    n_tiles = WPC * T_w
    S = n_tiles // ST

    gcut = np.searchsorted(batch, np.arange(0, N_GRAPHS + 1))
    cnt = np.diff(gcut).astype(np.float32)
    rec_all = (1.0 / np.maximum(cnt, 1.0)).astype(np.float32)

    x8 = x.astype(F8)
    rhi = r32.astype(F8)

    in_maps = []
    # shared constant blocks
    cb32_shared = np.concatenate([
        W_base[:, 0], W3[0, :], M2.reshape(-1), v0, W4[:, 0],
        [float(b_base.reshape(-1)[0])], [float(b4.reshape(-1)[0])],
    ]).astype(np.float32)
    pid_iot = np.concatenate([
        np.arange(128, dtype=np.float32).reshape(128, 1),       # pid
        np.broadcast_to(np.arange(128, dtype=np.float32)[None, :], (128, 128)),
    ], axis=1)

    for k in range(N_CORES):
        wx = np.zeros((WPC, T_w * 128, C), F8)
        wbr = np.full((WPC, T_w * 128), 255, np.uint8)
        for wi in range(WPC):
            gw = k * WPC + wi          # global window index
            s0, e0 = int(cut[gw]), int(cut[gw + 1])
            n = e0 - s0
            wx[wi, :n, 0:D] = x8[s0:e0]
            wx[wi, :n, D : D + R] = rhi[s0:e0]
            wbr[wi, :n] = (batch[s0:e0] - gw * W).astype(np.uint8)
        # window-interleaved tile order: tile i = s*ST + t belongs to window
        # i % WPC at within-window slot i // WPC; each SBUF partition line is
        # contiguous in DRAM.
        xcc_p = np.ascontiguousarray(
            wx.reshape(WPC, S, tpw, 128, C).transpose(1, 3, 2, 0, 4)
        ).reshape(S, 128, ST * C)
        brs_p = np.ascontiguousarray(
            wbr.reshape(WPC, S, tpw, 128).transpose(3, 1, 2, 0)
        ).reshape(128, S * ST)
        cb16_k = np.concatenate(
            [brs_p,
             np.broadcast_to(np.arange(W, dtype=np.uint8)[None, :], (128, W))],
            axis=1)
        rec_b = rec_all[k * GPC : (k + 1) * GPC]
        cb32_k = np.concatenate([
            np.broadcast_to(cb32_shared[None, :], (128, len(cb32_shared))),
            rec_b.reshape(128, 1),
            pid_iot,
        ], axis=1).astype(np.float32)
        in_maps.append(dict(xcc=xcc_p, cb16=np.ascontiguousarray(cb16_k),
                            cb32=np.ascontiguousarray(cb32_k)))

    nc = _build_program(S, C, R)

    kw = {}
    if _profile is not None:
        kw = dict(trace=True, **_profile)
    res = run_bass_kernel_spmd(nc, in_maps, list(range(N_CORES)), **kw)

    out = np.concatenate(
        [res.results[k]["out"].reshape(GPC, 1) for k in range(N_CORES)], axis=0)
    if _profile is not None:
        return out.astype(np.float32), res
    return out.astype(np.float32)
